# revision 3
# baseline (speedup 1.0000x reference)
"""DANet 3-layer MLP (B=8192, D=2048) on 8 Trainium2 NeuronCores.

Data-parallel: each core computes 1024 batch rows; weights replicated.
On-device layout is transposed (features on SBUF partitions) so every
matmul contracts over the partition dim and activations chain between
layers without transposes.

Arithmetic runs as fp8-e4m3 DoubleRow matmuls (2 packed contraction rows
per PE cell, half-rate per output column) with a hi/lo split for
accuracy: operands are decomposed v = hi + lo with hi = fp8(v) and
lo = fp8(v - hi), and each layer computes

    psum = x_hi @ W_hi + x_hi @ W_lo + x_lo @ W_hi        (W scaled by 32)

which carries ~fp16-level precision at 0.75x the PE cost of an exact
f32r chain (24 DoubleRow matmuls per 128x512 output tile vs 16 full-rate
f32r matmuls). The dropped lo*lo term is second order (~2^-13).

Weights are pre-quantized to fp8 hi/lo on the host (free); activations
are produced on-device: ACT computes a16 = tanh(psum/32 + b) in fp16,
ACT casts a_hi = fp8(a16), DVE computes a_lo = fp8(a16 - a_hi).
z/a outputs are stored as fp16 (halving store traffic) and upcast on the
host. Input x is shipped as fp8 hi/lo pairs (quarter traffic of f32).

DMA queues: loads on SP (sync), z-stores on DVE, a-stores on ACT, so no
DMA wait ever blocks a compute queue's decode stream.
"""

import numpy as np
import ml_dtypes

import concourse.mybir as mybir
import concourse.tile as tile
from concourse import bacc
from concourse.bass_utils import run_bass_kernel_spmd

NCORES = 8
B = 8192
D = 2048
BL = B // NCORES          # 1024 batch rows per core
P = 128                   # partitions
KT = D // P               # 16 contraction subtiles
NPANEL = 512              # weight-panel width (output features per panel)
NPB = D // NPANEL         # 4 panels per layer
NSB = NPANEL // P         # 4 output-feature subblocks per panel
MBLK = 512                # moving-operand width (batch cols per matmul)
MT = BL // MBLK           # 2 batch blocks
SCALE = 32.0              # weight pre-scale so fp8(W*32) uses normal range

f32 = mybir.dt.float32
f16 = mybir.dt.float16
fp8 = mybir.dt.float8e4
DR = mybir.MatmulPerfMode.DoubleRow
TANH = mybir.ActivationFunctionType.Tanh
COPY = mybir.ActivationFunctionType.Copy
MULT = mybir.AluOpType.mult
ADD = mybir.AluOpType.add
SUB = mybir.AluOpType.subtract
E4 = ml_dtypes.float8_e4m3fn

W_BUFS = 6                # weight slab pool ([128,16,512] fp8, 8KB/partition)


def build_nc():
    nc = bacc.Bacc()

    xhi_d = nc.dram_tensor("xhi", [D, BL], fp8, kind="ExternalInput")
    xlo_d = nc.dram_tensor("xlo", [D, BL], fp8, kind="ExternalInput")
    Whi_d = [nc.dram_tensor(f"Whi{l}", [D, D], fp8, kind="ExternalInput")
             for l in range(3)]
    Wlo_d = [nc.dram_tensor(f"Wlo{l}", [D, D], fp8, kind="ExternalInput")
             for l in range(3)]
    bs = [nc.dram_tensor(f"b{l}", [D], f32, kind="ExternalInput")
          for l in range(3)]
    zouts = [nc.dram_tensor(f"z{l}T", [D, BL], f16, kind="ExternalOutput")
             for l in range(3)]
    aouts = [nc.dram_tensor(f"a{l}T", [D, BL], f16, kind="ExternalOutput")
             for l in range(3)]

    with tile.TileContext(nc) as tc:
        with (
            tc.tile_pool(name="acts", bufs=1) as actp,
            tc.tile_pool(name="wpool", bufs=W_BUFS) as wpool,
            tc.tile_pool(name="zpool", bufs=4) as zpool,
            tc.tile_pool(name="apool", bufs=4) as apool,
            tc.tile_pool(name="misc", bufs=1) as misc,
            tc.tile_pool(name="psum", bufs=6, space="PSUM") as psp,
        ):
            # Persistent ping-pong activations, transposed fp8 hi/lo:
            # acts[s][h][p, k, c] = component h of a[feature 128k+p, col c].
            acts = [
                [actp.tile([P, KT, BL], fp8, name=f"act{s}_{h}", tag=f"act{s}_{h}")
                 for h in range(2)]
                for s in range(2)
            ]

            # All three biases in one [128, 48] tile; column l*16+c holds
            # b_l[128c : 128c+128].
            bias = misc.tile([P, 3 * KT], f32, name="bias", tag="bias")

            def load_x(h, m):
                ms = m * MBLK
                src = xhi_d if h == 0 else xlo_d
                nc.sync.dma_start(
                    acts[0][h][:, :, ms:ms + MBLK],
                    src[:, ms:ms + MBLK].rearrange("(k p) c -> p k c", p=P))

            def load_w(l, hl, nb):
                wt = wpool.tile([P, KT, NPANEL], fp8,
                                name=f"w_{l}_{hl}_{nb}", tag="wb")
                src = (Whi_d if hl == 0 else Wlo_d)[l]
                nc.sync.dma_start(
                    wt[:],
                    src[:, nb * NPANEL:(nb + 1) * NPANEL]
                    .rearrange("(k p) n -> p k n", p=P))
                return wt

            # Startup order follows the first chain's product order
            # (hi*hi, lo*hi, hi*lo): xhi-m0, Whi(0,0), xlo-m0, Wlo(0,0).
            load_x(0, 0)
            w00_hi = load_w(0, 0, 0)
            load_x(1, 0)
            w00_lo = load_w(0, 1, 0)
            for l in range(3):
                nc.sync.dma_start(
                    bias[:, l * KT:(l + 1) * KT],
                    bs[l][:].rearrange("(c p) -> p c", p=P))
            load_x(0, 1)
            load_x(1, 1)

            # Panel schedule with one-panel weight prefetch: panel idx+1's
            # loads are issued (on SP) before panel idx's chains, so the SP
            # queue never sits behind compute and transfers overlap the
            # previous panel's matmuls.
            panels = [(l, nb) for l in range(3) for nb in range(NPB)]
            slabs = {(0, 0): (w00_hi, w00_lo)}
            for idx, (l, nb) in enumerate(panels):
                if idx + 1 < len(panels):
                    ln, nbn = panels[idx + 1]
                    slabs[(ln, nbn)] = (load_w(ln, 0, nbn),
                                        load_w(ln, 1, nbn))
                whi_t, wlo_t = slabs.pop((l, nb))
                act_in = acts[l % 2]
                act_out = acts[(l + 1) % 2] if l < 2 else None
                # product order matches startup DMA arrival order
                prods = [(whi_t, 0), (whi_t, 1), (wlo_t, 0)]
                for ns in range(NSB):
                    ni = nb * NSB + ns
                    bcol = bias[:, l * KT + ni:l * KT + ni + 1]
                    z_t = zpool.tile([P, BL], f16,
                                     name=f"z_{l}_{ni}", tag="z16")
                    a_t = apool.tile([P, BL], f16,
                                     name=f"a_{l}_{ni}", tag="a16")
                    for m in range(MT):
                        ms = m * MBLK
                        psum = psp.tile([P, MBLK], f32,
                                        name=f"ps_{l}_{ni}_{m}", tag="ps")
                        n_mm = 3 * (KT // 2)
                        i = 0
                        for wt, h in prods:
                            for jp in range(KT // 2):
                                nc.tensor.matmul(
                                    psum[:],
                                    wt[:, 2 * jp:2 * jp + 2,
                                       ns * P:(ns + 1) * P],
                                    act_in[h][:, 2 * jp:2 * jp + 2,
                                              ms:ms + MBLK],
                                    start=(i == 0),
                                    stop=(i == n_mm - 1),
                                    perf_mode=DR)
                                i += 1
                        nc.vector.tensor_scalar(
                            z_t[:, ms:ms + MBLK], psum[:],
                            1.0 / SCALE, bcol, MULT, ADD)
                        nc.scalar.activation(
                            a_t[:, ms:ms + MBLK], psum[:], TANH,
                            bias=bcol, scale=1.0 / SCALE)
                    # next-layer hi/lo casts first (they gate the next
                    # layer's matmuls), then stores — all on the ACT queue,
                    # ordered so every DMA wait is satisfied at decode time
                    if act_out is not None:
                        nc.scalar.activation(act_out[0][:, ni, :], a_t[:],
                                             COPY)
                        nc.vector.tensor_tensor(act_out[1][:, ni, :],
                                                a_t[:],
                                                act_out[0][:, ni, :], SUB)
                    nc.scalar.dma_start(aouts[l][ni * P:(ni + 1) * P, :],
                                        a_t[:])
                    nc.scalar.dma_start(zouts[l][ni * P:(ni + 1) * P, :],
                                        z_t[:])

    nc.finalize()
    return nc


_NC_CACHE = None


def _get_nc():
    global _NC_CACHE
    if _NC_CACHE is None:
        _NC_CACHE = build_nc()
    return _NC_CACHE


def _hilo(v):
    hi = v.astype(E4)
    lo = (v - hi.astype(np.float32)).astype(E4)
    return hi, lo


def make_in_maps(x, W0, b0, W1, b1, W2, b2):
    weights = {}
    for l, W in enumerate((W0, W1, W2)):
        hi, lo = _hilo(np.asarray(W, dtype=np.float32) * SCALE)
        weights[f"Whi{l}"] = hi
        weights[f"Wlo{l}"] = lo
    for l, b in enumerate((b0, b1, b2)):
        weights[f"b{l}"] = np.asarray(b, dtype=np.float32)
    x = np.asarray(x, dtype=np.float32)
    in_maps = []
    for c in range(NCORES):
        xT = np.ascontiguousarray(x[c * BL:(c + 1) * BL, :].T)
        xhi, xlo = _hilo(xT)
        in_maps.append({"xhi": xhi, "xlo": xlo, **weights})
    return in_maps


def kernel(x, W0, b0, W1, b1, W2, b2):
    in_maps = make_in_maps(x, W0, b0, W1, b1, W2, b2)
    res = run_bass_kernel_spmd(_get_nc(), in_maps, core_ids=list(range(NCORES)))

    out = np.empty((6, B, D), dtype=np.float32)
    for c in range(NCORES):
        r = res.results[c]
        rows = slice(c * BL, (c + 1) * BL)
        for l in range(3):
            out[l, rows, :] = r[f"z{l}T"].astype(np.float32).T
            out[3 + l, rows, :] = r[f"a{l}T"].astype(np.float32).T
    return out


# revision 19
# speedup vs baseline: 1.0533x; 1.0533x over previous
"""DANet 3-layer MLP (B=8192, D=2048) on 8 Trainium2 NeuronCores.

Data-parallel: each core computes 1024 batch rows; weights replicated.
On-device layout is transposed (features on SBUF partitions) so every
matmul contracts over the partition dim and activations chain between
layers without transposes.

Arithmetic runs as fp8-e4m3 DoubleRow matmuls (2 packed contraction rows
per PE cell, half-rate per output column) with a hi/lo split for
accuracy: operands are decomposed v = hi + lo with hi = fp8(v) and
lo = fp8(v - hi), and each layer computes

    psum = x_hi @ W_hi + x_lo @ W_hi + x_hi @ W_lo       (W scaled by 32)

which carries ~fp16-level precision at 0.75x the PE cost of an exact
f32r chain (24 DoubleRow matmuls per 128x512 output tile vs 16 full-rate
f32r matmuls). The dropped lo*lo term is second order (~2^-13).

Weights are pre-quantized to fp8 hi/lo on the host and laid out in DRAM
as [panel][subblock][partition][k][128] so both full-panel slabs and the
startup-critical per-subblock chunks load with 2 KiB contiguous runs.
Activations are produced on-device: ACT computes a16 = tanh(psum/32 + b)
in fp16, ACT casts a_hi = fp8(a16), DVE computes a_lo = fp8(a16 - a_hi).
z/a outputs are stored as fp16 (half the store traffic) and upcast on
the host. Input x ships as fp8 hi/lo pairs (quarter traffic of f32).

Schedule: weight loads prefetch one panel ahead on the SP queue (which
never waits on compute); stores and casts ride the ACT queue ordered so
every DMA wait is already satisfied at decode time. The first panel runs
m-outer with k-chunked x loads so the PE starts ~4us into the kernel;
the last panel stores half-tiles per m-block to shorten the tail.
"""

import numpy as np
import ml_dtypes

import concourse.mybir as mybir
import concourse.tile as tile
from concourse import bacc
from concourse.bass_utils import run_bass_kernel_spmd

NCORES = 8
B = 8192
D = 2048
BL = B // NCORES          # 1024 batch rows per core
P = 128                   # partitions
KT = D // P               # 16 contraction subtiles
NPANEL = 512              # weight-panel width (output features per panel)
NPB = D // NPANEL         # 4 panels per layer
NSB = NPANEL // P         # 4 output-feature subblocks per panel
MBLK = 512                # moving-operand width (batch cols per matmul)
MT = BL // MBLK           # 2 batch blocks
SCALE = 32.0              # weight pre-scale so fp8(W*32) uses normal range

f32 = mybir.dt.float32
f16 = mybir.dt.float16
fp8 = mybir.dt.float8e4
DR = mybir.MatmulPerfMode.DoubleRow
TANH = mybir.ActivationFunctionType.Tanh
COPY = mybir.ActivationFunctionType.Copy
MULT = mybir.AluOpType.mult
ADD = mybir.AluOpType.add
SUB = mybir.AluOpType.subtract
E4 = ml_dtypes.float8_e4m3fn

W_BUFS = 6                # weight slab pool ([128,4,16,128] fp8, 8KB/partition)


def build_nc():
    nc = bacc.Bacc()

    xhi_d = nc.dram_tensor("xhi", [D, BL], fp8, kind="ExternalInput")
    xlo_d = nc.dram_tensor("xlo", [D, BL], fp8, kind="ExternalInput")
    # [nb, ns, p, k, n]: W[128k+p, 512nb+128ns+n] * 32, quantized hi/lo
    Whi_d = [nc.dram_tensor(f"Whi{l}", [NPB, NSB, P, KT, P], fp8,
                            kind="ExternalInput") for l in range(3)]
    Wlo_d = [nc.dram_tensor(f"Wlo{l}", [NPB, NSB, P, KT, P], fp8,
                            kind="ExternalInput") for l in range(3)]
    bs = [nc.dram_tensor(f"b{l}", [D], f32, kind="ExternalInput")
          for l in range(3)]
    zouts = [nc.dram_tensor(f"z{l}T", [D, BL], f16, kind="ExternalOutput")
             for l in range(3)]
    aouts = [nc.dram_tensor(f"a{l}T", [D, BL], f16, kind="ExternalOutput")
             for l in range(3)]

    with tile.TileContext(nc) as tc:
        with (
            tc.tile_pool(name="acts", bufs=1) as actp,
            tc.tile_pool(name="wpool", bufs=W_BUFS) as wpool,
            tc.tile_pool(name="zpool", bufs=4) as zpool,
            tc.tile_pool(name="apool", bufs=4) as apool,
            tc.tile_pool(name="misc", bufs=1) as misc,
            tc.tile_pool(name="psum", bufs=6, space="PSUM") as psp,
        ):
            # Persistent ping-pong activations, transposed fp8 hi/lo:
            # acts[s][h][p, k, c] = component h of a[feature 128k+p, col c].
            acts = [
                [actp.tile([P, KT, BL], fp8, name=f"act{s}_{h}", tag=f"act{s}_{h}")
                 for h in range(2)]
                for s in range(2)
            ]

            # All three biases in one [128, 48] tile; column l*16+c holds
            # b_l[128c : 128c+128].
            bias = misc.tile([P, 3 * KT], f32, name="bias", tag="bias")

            def load_x(h, m):
                ms = m * MBLK
                src = xhi_d if h == 0 else xlo_d
                nc.sync.dma_start(
                    acts[0][h][:, :, ms:ms + MBLK],
                    src[:, ms:ms + MBLK].rearrange("(k p) c -> p k c", p=P))

            def load_x_chunk(h, m, k0, k1):
                # k-subtiles [k0, k1) of batch half m — startup granularity
                ms = m * MBLK
                src = xhi_d if h == 0 else xlo_d
                nc.sync.dma_start(
                    acts[0][h][:, k0:k1, ms:ms + MBLK],
                    src[k0 * P:k1 * P, ms:ms + MBLK]
                    .rearrange("(k p) c -> p k c", p=P))

            def w_tile(l, hl, nb):
                return wpool.tile([P, NSB, KT, P], fp8,
                                  name=f"w_{l}_{hl}_{nb}", tag="wb")

            def load_w(l, hl, nb):
                wt = w_tile(l, hl, nb)
                src = (Whi_d if hl == 0 else Wlo_d)[l]
                nc.sync.dma_start(
                    wt[:], src[nb].rearrange("ns p k n -> p ns k n"))
                return wt

            def load_w_chunk(wt, l, hl, nb, ns):
                src = (Whi_d if hl == 0 else Wlo_d)[l]
                nc.sync.dma_start(wt[:, ns], src[nb, ns])

            # --- startup: critical-path-ordered loads -------------------
            # First chain (panel 0, ns0, m0) consumes x-m0 + W(0,0) ns0 in
            # product order hi*hi, hi*lo, lo*hi; interleave loads in exactly
            # the consumption order so the PE starts ~3us in and never
            # starves for long. x-m1 and panel (0,1) follow.
            w00_hi = w_tile(0, 0, 0)
            w00_lo = w_tile(0, 1, 0)
            load_x_chunk(0, 0, 0, 4)
            load_w_chunk(w00_hi, 0, 0, 0, 0)
            load_x_chunk(0, 0, 4, 10)
            load_w_chunk(w00_lo, 0, 1, 0, 0)
            load_x_chunk(0, 0, 10, 16)
            load_x_chunk(1, 0, 0, 8)
            load_x_chunk(1, 0, 8, 16)
            for ns in range(1, NSB):
                load_w_chunk(w00_hi, 0, 0, 0, ns)
                load_w_chunk(w00_lo, 0, 1, 0, ns)
            for l in range(3):
                nc.sync.dma_start(
                    bias[:, l * KT:(l + 1) * KT],
                    bs[l][:].rearrange("(c p) -> p c", p=P))
            load_x(0, 1)
            load_x(1, 1)

            # --- panel schedule with one-panel weight prefetch ----------
            panels = [(l, nb) for l in range(3) for nb in range(NPB)]
            slabs = {(0, 0): (w00_hi, w00_lo)}
            for idx, (l, nb) in enumerate(panels):
                if idx + 1 < len(panels):
                    ln, nbn = panels[idx + 1]
                    slabs[(ln, nbn)] = (load_w(ln, 0, nbn),
                                        load_w(ln, 1, nbn))
                whi_t, wlo_t = slabs.pop((l, nb))
                act_in = acts[l % 2]
                act_out = acts[(l + 1) % 2] if l < 2 else None
                last_panel = idx == len(panels) - 1
                # product order matches startup DMA arrival order
                prods = [(whi_t, 0), (wlo_t, 0), (whi_t, 1)]

                z_ts, a_ts = {}, {}
                for ns in range(NSB):
                    ni = nb * NSB + ns
                    z_ts[ns] = zpool.tile([P, BL], f16,
                                          name=f"z_{l}_{ni}", tag="z16")
                    a_ts[ns] = apool.tile([P, BL], f16,
                                          name=f"a_{l}_{ni}", tag="a16")

                # first panel: m-outer so the m1 chains (whose x arrives
                # last) run after all four m0 chains; last panel: quarter
                # blocks so the tail drains in 256-col steps
                if idx == 0:
                    order = [(ns, m * MBLK, MBLK)
                             for m in range(MT) for ns in range(NSB)]
                elif last_panel:
                    order = ([(ns, m * MBLK, MBLK)
                              for ns in range(NSB - 1) for m in range(MT)]
                             + [(NSB - 1, q * (MBLK // 2), MBLK // 2)
                                for q in range(2 * MT)])
                else:
                    order = [(ns, m * MBLK, MBLK)
                             for ns in range(NSB) for m in range(MT)]

                for ns, ms, mw in order:
                    ni = nb * NSB + ns
                    bcol = bias[:, l * KT + ni:l * KT + ni + 1]
                    z_t, a_t = z_ts[ns], a_ts[ns]
                    psum = psp.tile([P, MBLK], f32,
                                    name=f"ps_{l}_{ni}_{ms}", tag="ps")
                    n_mm = 3 * (KT // 2)
                    i = 0
                    for wt, h in prods:
                        for jp in range(KT // 2):
                            nc.tensor.matmul(
                                psum[:, :mw],
                                wt[:, ns, 2 * jp:2 * jp + 2, :],
                                act_in[h][:, 2 * jp:2 * jp + 2,
                                          ms:ms + mw],
                                start=(i == 0),
                                stop=(i == n_mm - 1),
                                perf_mode=DR)
                            i += 1
                    nc.vector.tensor_scalar(
                        z_t[:, ms:ms + mw], psum[:, :mw],
                        1.0 / SCALE, bcol, MULT, ADD)
                    nc.scalar.activation(
                        a_t[:, ms:ms + mw], psum[:, :mw], TANH,
                        bias=bcol, scale=1.0 / SCALE)
                    if last_panel:
                        # tail: store blocks immediately, z and a on
                        # different queues so the final two stores overlap
                        # their fixed DGE/sem latencies
                        nc.scalar.dma_start(
                            zouts[l][ni * P:(ni + 1) * P, ms:ms + mw],
                            z_t[:, ms:ms + mw])
                        nc.sync.dma_start(
                            aouts[l][ni * P:(ni + 1) * P, ms:ms + mw],
                            a_t[:, ms:ms + mw])
                        continue
                    if ms + mw == BL:
                        # next-layer hi/lo casts first (they gate the next
                        # layer's matmuls), then stores — all on the ACT
                        # queue, ordered so every DMA wait is satisfied at
                        # decode time
                        if act_out is not None:
                            nc.scalar.activation(act_out[0][:, ni, :],
                                                 a_t[:], COPY)
                            nc.vector.tensor_tensor(act_out[1][:, ni, :],
                                                    a_t[:],
                                                    act_out[0][:, ni, :],
                                                    SUB)
                        nc.scalar.dma_start(
                            aouts[l][ni * P:(ni + 1) * P, :], a_t[:])
                        nc.scalar.dma_start(
                            zouts[l][ni * P:(ni + 1) * P, :], z_t[:])

    nc.finalize()
    return nc


_NC_CACHE = None


def _get_nc():
    global _NC_CACHE
    if _NC_CACHE is None:
        _NC_CACHE = build_nc()
    return _NC_CACHE


def _hilo(v):
    hi = v.astype(E4)
    lo = (v - hi.astype(np.float32)).astype(E4)
    return hi, lo


def _w_layout(w8):
    # [D, D] -> [nb, ns, p, k, n] with W[128k+p, 512nb+128ns+n]
    return np.ascontiguousarray(
        w8.reshape(KT, P, NPB, NSB, P).transpose(2, 3, 1, 0, 4))


def make_in_maps(x, W0, b0, W1, b1, W2, b2):
    weights = {}
    for l, W in enumerate((W0, W1, W2)):
        hi, lo = _hilo(np.asarray(W, dtype=np.float32) * SCALE)
        weights[f"Whi{l}"] = _w_layout(hi)
        weights[f"Wlo{l}"] = _w_layout(lo)
    for l, b in enumerate((b0, b1, b2)):
        weights[f"b{l}"] = np.asarray(b, dtype=np.float32)
    x = np.asarray(x, dtype=np.float32)
    in_maps = []
    for c in range(NCORES):
        xT = np.ascontiguousarray(x[c * BL:(c + 1) * BL, :].T)
        xhi, xlo = _hilo(xT)
        in_maps.append({"xhi": xhi, "xlo": xlo, **weights})
    return in_maps


def kernel(x, W0, b0, W1, b1, W2, b2):
    in_maps = make_in_maps(x, W0, b0, W1, b1, W2, b2)
    res = run_bass_kernel_spmd(_get_nc(), in_maps, core_ids=list(range(NCORES)))

    out = np.empty((6, B, D), dtype=np.float32)
    for c in range(NCORES):
        r = res.results[c]
        rows = slice(c * BL, (c + 1) * BL)
        for l in range(3):
            out[l, rows, :] = r[f"z{l}T"].astype(np.float32).T
            out[3 + l, rows, :] = r[f"a{l}T"].astype(np.float32).T
    return out


# revision 27
# speedup vs baseline: 1.0541x; 1.0008x over previous
"""DANet 3-layer MLP (B=8192, D=2048) on 8 Trainium2 NeuronCores.

Data-parallel: each core computes 1024 batch rows; weights replicated.
On-device layout is transposed (features on SBUF partitions) so every
matmul contracts over the partition dim and activations chain between
layers without transposes.

Arithmetic runs as fp8-e4m3 DoubleRow matmuls (2 packed contraction rows
per PE cell, half-rate per output column) with a hi/lo split for
accuracy: operands are decomposed v = hi + lo with hi = fp8(v) and
lo = fp8(v - hi), and each layer computes

    psum = x_hi @ W_hi + x_lo @ W_hi + x_hi @ W_lo       (W scaled by 32)

which carries ~fp16-level precision at 0.75x the PE cost of an exact
f32r chain (24 DoubleRow matmuls per 128x512 output tile vs 16 full-rate
f32r matmuls). The dropped lo*lo term is second order (~2^-13).

Weights are pre-quantized to fp8 hi/lo on the host and laid out in DRAM
as [panel][subblock][partition][k][128] so both full-panel slabs and the
startup-critical per-subblock chunks load with 2 KiB contiguous runs.
Activations are produced on-device: ACT computes a16 = tanh(psum/32 + b)
in fp16, ACT casts a_hi = fp8(a16), DVE computes a_lo = fp8(a16 - a_hi).
z/a outputs are stored as fp16 (half the store traffic) and upcast on
the host. Input x ships as fp8 hi/lo pairs (quarter traffic of f32).

Schedule: weight loads prefetch one panel ahead on the SP queue (which
never waits on compute); stores and casts ride the ACT queue ordered so
every DMA wait is already satisfied at decode time. The first panel runs
m-outer with k-chunked x loads so the PE starts ~4us into the kernel;
the last panel stores half-tiles per m-block to shorten the tail.
"""

import numpy as np
import ml_dtypes

import concourse.mybir as mybir
import concourse.tile as tile
from concourse import bacc
from concourse.bass_utils import run_bass_kernel_spmd

NCORES = 8
B = 8192
D = 2048
BL = B // NCORES          # 1024 batch rows per core
P = 128                   # partitions
KT = D // P               # 16 contraction subtiles
NPANEL = 512              # weight-panel width (output features per panel)
NPB = D // NPANEL         # 4 panels per layer
NSB = NPANEL // P         # 4 output-feature subblocks per panel
MBLK = 512                # moving-operand width (batch cols per matmul)
MT = BL // MBLK           # 2 batch blocks
SCALE = 32.0              # weight pre-scale so fp8(W*32) uses normal range

f32 = mybir.dt.float32
f16 = mybir.dt.float16
fp8 = mybir.dt.float8e4
DR = mybir.MatmulPerfMode.DoubleRow
TANH = mybir.ActivationFunctionType.Tanh
COPY = mybir.ActivationFunctionType.Copy
MULT = mybir.AluOpType.mult
ADD = mybir.AluOpType.add
SUB = mybir.AluOpType.subtract
E4 = ml_dtypes.float8_e4m3fn

W_BUFS = 6                # weight slab pool ([128,4,16,128] fp8, 8KB/partition)


def build_nc():
    nc = bacc.Bacc()

    xhi_d = nc.dram_tensor("xhi", [D, BL], fp8, kind="ExternalInput")
    xlo_d = nc.dram_tensor("xlo", [D, BL], fp8, kind="ExternalInput")
    # [nb, ns, p, k, n]: W[128k+p, 512nb+128ns+n] * 32, quantized hi/lo
    Whi_d = [nc.dram_tensor(f"Whi{l}", [NPB, NSB, P, KT, P], fp8,
                            kind="ExternalInput") for l in range(3)]
    Wlo_d = [nc.dram_tensor(f"Wlo{l}", [NPB, NSB, P, KT, P], fp8,
                            kind="ExternalInput") for l in range(3)]
    bs = [nc.dram_tensor(f"b{l}", [D], f32, kind="ExternalInput")
          for l in range(3)]
    zouts = [nc.dram_tensor(f"z{l}T", [D, BL], f16, kind="ExternalOutput")
             for l in range(3)]
    aouts = [nc.dram_tensor(f"a{l}T", [D, BL], f16, kind="ExternalOutput")
             for l in range(3)]

    with tile.TileContext(nc) as tc:
        with (
            tc.tile_pool(name="acts", bufs=1) as actp,
            tc.tile_pool(name="wpool", bufs=W_BUFS) as wpool,
            tc.tile_pool(name="zpool", bufs=4) as zpool,
            tc.tile_pool(name="apool", bufs=4) as apool,
            tc.tile_pool(name="misc", bufs=1) as misc,
            tc.tile_pool(name="psum", bufs=6, space="PSUM") as psp,
        ):
            # Persistent ping-pong activations, transposed fp8 hi/lo:
            # acts[s][h][p, k, c] = component h of a[feature 128k+p, col c].
            acts = [
                [actp.tile([P, KT, BL], fp8, name=f"act{s}_{h}", tag=f"act{s}_{h}")
                 for h in range(2)]
                for s in range(2)
            ]

            # All three biases in one [128, 48] tile; column l*16+c holds
            # b_l[128c : 128c+128].
            bias = misc.tile([P, 3 * KT], f32, name="bias", tag="bias")

            def load_x(h, m):
                ms = m * MBLK
                src = xhi_d if h == 0 else xlo_d
                nc.sync.dma_start(
                    acts[0][h][:, :, ms:ms + MBLK],
                    src[:, ms:ms + MBLK].rearrange("(k p) c -> p k c", p=P))

            def load_x_chunk(h, m, k0, k1):
                # k-subtiles [k0, k1) of batch half m — startup granularity
                ms = m * MBLK
                src = xhi_d if h == 0 else xlo_d
                nc.sync.dma_start(
                    acts[0][h][:, k0:k1, ms:ms + MBLK],
                    src[k0 * P:k1 * P, ms:ms + MBLK]
                    .rearrange("(k p) c -> p k c", p=P))

            def w_tile(l, hl, nb):
                return wpool.tile([P, NSB, KT, P], fp8,
                                  name=f"w_{l}_{hl}_{nb}", tag="wb")

            def load_w(l, hl, nb):
                wt = w_tile(l, hl, nb)
                src = (Whi_d if hl == 0 else Wlo_d)[l]
                nc.sync.dma_start(
                    wt[:], src[nb].rearrange("ns p k n -> p ns k n"))
                return wt

            def load_w_chunk(wt, l, hl, nb, ns):
                src = (Whi_d if hl == 0 else Wlo_d)[l]
                nc.sync.dma_start(wt[:, ns], src[nb, ns])

            # --- startup: critical-path-ordered loads -------------------
            # First chain (panel 0, ns0, m0) consumes x-m0 + W(0,0) ns0 in
            # product order hi*hi, hi*lo, lo*hi; interleave loads in exactly
            # the consumption order so the PE starts ~3us in and never
            # starves for long. x-m1 and panel (0,1) follow.
            w00_hi = w_tile(0, 0, 0)
            w00_lo = w_tile(0, 1, 0)
            load_x_chunk(0, 0, 0, 4)
            load_w_chunk(w00_hi, 0, 0, 0, 0)
            load_x_chunk(0, 0, 4, 10)
            load_w_chunk(w00_lo, 0, 1, 0, 0)
            load_x_chunk(0, 0, 10, 16)
            load_x_chunk(1, 0, 0, 8)
            load_x_chunk(1, 0, 8, 16)
            for ns in range(1, NSB):
                load_w_chunk(w00_hi, 0, 0, 0, ns)
                load_w_chunk(w00_lo, 0, 1, 0, ns)
            for l in range(3):
                nc.sync.dma_start(
                    bias[:, l * KT:(l + 1) * KT],
                    bs[l][:].rearrange("(c p) -> p c", p=P))
            load_x(0, 1)
            load_x(1, 1)

            # --- panel schedule with one-panel weight prefetch ----------
            panels = [(l, nb) for l in range(3) for nb in range(NPB)]
            slabs = {(0, 0): (w00_hi, w00_lo)}
            for idx, (l, nb) in enumerate(panels):
                if idx + 1 < len(panels):
                    ln, nbn = panels[idx + 1]
                    slabs[(ln, nbn)] = (load_w(ln, 0, nbn),
                                        load_w(ln, 1, nbn))
                whi_t, wlo_t = slabs.pop((l, nb))
                act_in = acts[l % 2]
                act_out = acts[(l + 1) % 2] if l < 2 else None
                last_panel = idx == len(panels) - 1
                # product order matches startup DMA arrival order
                prods = [(whi_t, 0), (wlo_t, 0), (whi_t, 1)]

                z_ts, a_ts = {}, {}
                for ns in range(NSB):
                    ni = nb * NSB + ns
                    z_ts[ns] = zpool.tile([P, BL], f16,
                                          name=f"z_{l}_{ni}", tag="z16")
                    a_ts[ns] = apool.tile([P, BL], f16,
                                          name=f"a_{l}_{ni}", tag="a16")

                # first panel: m-outer so the m1 chains (whose x arrives
                # last) run after all four m0 chains; last panel: quarter
                # blocks so the tail drains in 256-col steps
                if idx == 0:
                    order = [(ns, m * MBLK, MBLK)
                             for m in range(MT) for ns in range(NSB)]
                elif last_panel:
                    order = ([(ns, m * MBLK, MBLK)
                              for ns in range(NSB - 1) for m in range(MT)]
                             + [(NSB - 1, q * (MBLK // 2), MBLK // 2)
                                for q in range(2 * MT)])
                else:
                    order = [(ns, q * (MBLK // 2), MBLK // 2)
                             for ns in range(NSB) for q in range(2 * MT)]

                for ns, ms, mw in order:
                    ni = nb * NSB + ns
                    bcol = bias[:, l * KT + ni:l * KT + ni + 1]
                    z_t, a_t = z_ts[ns], a_ts[ns]
                    psum = psp.tile([P, MBLK], f32,
                                    name=f"ps_{l}_{ni}_{ms}", tag="ps")
                    n_mm = 3 * (KT // 2)
                    i = 0
                    for wt, h in prods:
                        for jp in range(KT // 2):
                            nc.tensor.matmul(
                                psum[:, :mw],
                                wt[:, ns, 2 * jp:2 * jp + 2, :],
                                act_in[h][:, 2 * jp:2 * jp + 2,
                                          ms:ms + mw],
                                start=(i == 0),
                                stop=(i == n_mm - 1),
                                perf_mode=DR)
                            i += 1
                    nc.vector.tensor_scalar(
                        z_t[:, ms:ms + mw], psum[:, :mw],
                        1.0 / SCALE, bcol, MULT, ADD)
                    nc.scalar.activation(
                        a_t[:, ms:ms + mw], psum[:, :mw], TANH,
                        bias=bcol, scale=1.0 / SCALE)
                    if last_panel:
                        # tail: store blocks immediately. ns0-2 ride the
                        # Pool SWDGE path (no HWDGE slot), keeping HWDGE
                        # free so the final ns3 stores launch with minimal
                        # latency on the ACT/SP hardware-DGE queues.
                        if ns < NSB - 1:
                            nc.gpsimd.dma_start(
                                zouts[l][ni * P:(ni + 1) * P, ms:ms + mw],
                                z_t[:, ms:ms + mw])
                            nc.gpsimd.dma_start(
                                aouts[l][ni * P:(ni + 1) * P, ms:ms + mw],
                                a_t[:, ms:ms + mw])
                        else:
                            nc.scalar.dma_start(
                                zouts[l][ni * P:(ni + 1) * P, ms:ms + mw],
                                z_t[:, ms:ms + mw])
                            nc.sync.dma_start(
                                aouts[l][ni * P:(ni + 1) * P, ms:ms + mw],
                                a_t[:, ms:ms + mw])
                        continue
                    if ms + mw == BL:
                        # next-layer hi/lo casts first (they gate the next
                        # layer's matmuls), then stores — all on the ACT
                        # queue, ordered so every DMA wait is satisfied at
                        # decode time
                        if act_out is not None:
                            nc.scalar.activation(act_out[0][:, ni, :],
                                                 a_t[:], COPY)
                            nc.vector.tensor_tensor(act_out[1][:, ni, :],
                                                    a_t[:],
                                                    act_out[0][:, ni, :],
                                                    SUB)
                        nc.scalar.dma_start(
                            aouts[l][ni * P:(ni + 1) * P, :], a_t[:])
                        nc.scalar.dma_start(
                            zouts[l][ni * P:(ni + 1) * P, :], z_t[:])

    nc.finalize()
    return nc


_NC_CACHE = None


def _get_nc():
    global _NC_CACHE
    if _NC_CACHE is None:
        _NC_CACHE = build_nc()
    return _NC_CACHE


def _hilo(v):
    hi = v.astype(E4)
    lo = (v - hi.astype(np.float32)).astype(E4)
    return hi, lo


def _w_layout(w8):
    # [D, D] -> [nb, ns, p, k, n] with W[128k+p, 512nb+128ns+n]
    return np.ascontiguousarray(
        w8.reshape(KT, P, NPB, NSB, P).transpose(2, 3, 1, 0, 4))


def make_in_maps(x, W0, b0, W1, b1, W2, b2):
    weights = {}
    for l, W in enumerate((W0, W1, W2)):
        hi, lo = _hilo(np.asarray(W, dtype=np.float32) * SCALE)
        weights[f"Whi{l}"] = _w_layout(hi)
        weights[f"Wlo{l}"] = _w_layout(lo)
    for l, b in enumerate((b0, b1, b2)):
        weights[f"b{l}"] = np.asarray(b, dtype=np.float32)
    x = np.asarray(x, dtype=np.float32)
    in_maps = []
    for c in range(NCORES):
        xT = np.ascontiguousarray(x[c * BL:(c + 1) * BL, :].T)
        xhi, xlo = _hilo(xT)
        in_maps.append({"xhi": xhi, "xlo": xlo, **weights})
    return in_maps


def kernel(x, W0, b0, W1, b1, W2, b2):
    in_maps = make_in_maps(x, W0, b0, W1, b1, W2, b2)
    res = run_bass_kernel_spmd(_get_nc(), in_maps, core_ids=list(range(NCORES)))

    out = np.empty((6, B, D), dtype=np.float32)
    for c in range(NCORES):
        r = res.results[c]
        rows = slice(c * BL, (c + 1) * BL)
        for l in range(3):
            out[l, rows, :] = r[f"z{l}T"].astype(np.float32).T
            out[3 + l, rows, :] = r[f"a{l}T"].astype(np.float32).T
    return out


# revision 35
# speedup vs baseline: 1.0620x; 1.0074x over previous
"""DANet 3-layer MLP (B=8192, D=2048) on 8 Trainium2 NeuronCores.

Data-parallel: each core computes 1024 batch rows; weights replicated.
On-device layout is transposed (features on SBUF partitions) so every
matmul contracts over the partition dim and activations chain between
layers without transposes.

Arithmetic runs as fp8-e4m3 DoubleRow matmuls (2 packed contraction rows
per PE cell, half-rate per output column) with a hi/lo split for
accuracy: operands are decomposed v = hi + lo with hi = fp8(v) and
lo = fp8(v - hi), and each layer computes

    psum = x_hi @ W_hi + x_lo @ W_hi + x_hi @ W_lo       (W scaled by 32)

which carries ~fp16-level precision at 0.75x the PE cost of an exact
f32r chain (24 DoubleRow matmuls per 128x512 output tile vs 16 full-rate
f32r matmuls). The dropped lo*lo term is second order (~2^-13).

Weights are pre-quantized to fp8 hi/lo on the host and laid out in DRAM
as [panel][subblock][partition][k][128] so both full-panel slabs and the
startup-critical per-subblock chunks load with 2 KiB contiguous runs.
Activations are produced on-device: ACT computes a16 = tanh(psum/32 + b)
in fp16, ACT casts a_hi = fp8(a16), DVE computes a_lo = fp8(a16 - a_hi).
z/a outputs are stored as fp16 (half the store traffic) and upcast on
the host. Input x ships as fp8 hi/lo pairs (quarter traffic of f32).

Schedule: weight loads prefetch one panel ahead on the SP queue (which
never waits on compute); stores and casts ride the ACT queue ordered so
every DMA wait is already satisfied at decode time. The first panel runs
m-outer with k-chunked x loads so the PE starts ~4us into the kernel.
Steady-state chains use 256-col blocks (53ns/matmul vs 107 for 512 after
integer rounding of the per-instruction delay) across 8 PSUM banks; the
last panel's bulk stores ride the Pool SWDGE path to keep HWDGE free so
the final stores launch with minimal latency.
"""

import numpy as np
import ml_dtypes

import concourse.mybir as mybir
import concourse.tile as tile
from concourse import bacc
from concourse.bass_utils import run_bass_kernel_spmd

NCORES = 8
B = 8192
D = 2048
BL = B // NCORES          # 1024 batch rows per core
P = 128                   # partitions
KT = D // P               # 16 contraction subtiles
NPANEL = 512              # weight-panel width (output features per panel)
NPB = D // NPANEL         # 4 panels per layer
NSB = NPANEL // P         # 4 output-feature subblocks per panel
MBLK = 512                # moving-operand width (batch cols per matmul)
MT = BL // MBLK           # 2 batch blocks
SCALE = 32.0              # weight pre-scale so fp8(W*32) uses normal range

f32 = mybir.dt.float32
f16 = mybir.dt.float16
fp8 = mybir.dt.float8e4
DR = mybir.MatmulPerfMode.DoubleRow
TANH = mybir.ActivationFunctionType.Tanh
COPY = mybir.ActivationFunctionType.Copy
MULT = mybir.AluOpType.mult
ADD = mybir.AluOpType.add
SUB = mybir.AluOpType.subtract
E4 = ml_dtypes.float8_e4m3fn

W_BUFS = 6                # weight slab pool ([128,4,16,128] fp8, 8KB/partition)


def build_nc():
    nc = bacc.Bacc()

    xhi_d = nc.dram_tensor("xhi", [D, BL], fp8, kind="ExternalInput")
    xlo_d = nc.dram_tensor("xlo", [D, BL], fp8, kind="ExternalInput")
    # [nb, ns, p, k, n]: W[128k+p, 512nb+128ns+n] * 32, quantized hi/lo
    Whi_d = [nc.dram_tensor(f"Whi{l}", [NPB, NSB, P, KT, P], fp8,
                            kind="ExternalInput") for l in range(3)]
    Wlo_d = [nc.dram_tensor(f"Wlo{l}", [NPB, NSB, P, KT, P], fp8,
                            kind="ExternalInput") for l in range(3)]
    bs = [nc.dram_tensor(f"b{l}", [D], f32, kind="ExternalInput")
          for l in range(3)]
    zouts = [nc.dram_tensor(f"z{l}T", [D, BL], f16, kind="ExternalOutput")
             for l in range(3)]
    aouts = [nc.dram_tensor(f"a{l}T", [D, BL], f16, kind="ExternalOutput")
             for l in range(3)]

    with tile.TileContext(nc) as tc:
        with (
            tc.tile_pool(name="acts", bufs=1) as actp,
            tc.tile_pool(name="wpool", bufs=W_BUFS) as wpool,
            tc.tile_pool(name="zpool", bufs=4) as zpool,
            tc.tile_pool(name="apool", bufs=4) as apool,
            tc.tile_pool(name="misc", bufs=1) as misc,
            tc.tile_pool(name="psum", bufs=8, space="PSUM") as psp,
        ):
            # Persistent ping-pong activations, transposed fp8 hi/lo:
            # acts[s][h][p, k, c] = component h of a[feature 128k+p, col c].
            acts = [
                [actp.tile([P, KT, BL], fp8, name=f"act{s}_{h}", tag=f"act{s}_{h}")
                 for h in range(2)]
                for s in range(2)
            ]

            # All three biases in one [128, 48] tile; column l*16+c holds
            # b_l[128c : 128c+128].
            bias = misc.tile([P, 3 * KT], f32, name="bias", tag="bias")

            def load_x(h, m):
                ms = m * MBLK
                src = xhi_d if h == 0 else xlo_d
                nc.sync.dma_start(
                    acts[0][h][:, :, ms:ms + MBLK],
                    src[:, ms:ms + MBLK].rearrange("(k p) c -> p k c", p=P))

            def load_x_chunk(h, m, k0, k1):
                # k-subtiles [k0, k1) of batch half m — startup granularity
                ms = m * MBLK
                src = xhi_d if h == 0 else xlo_d
                nc.sync.dma_start(
                    acts[0][h][:, k0:k1, ms:ms + MBLK],
                    src[k0 * P:k1 * P, ms:ms + MBLK]
                    .rearrange("(k p) c -> p k c", p=P))

            def w_tile(l, hl, nb):
                return wpool.tile([P, NSB, KT, P], fp8,
                                  name=f"w_{l}_{hl}_{nb}", tag="wb")

            def load_w(l, hl, nb):
                wt = w_tile(l, hl, nb)
                src = (Whi_d if hl == 0 else Wlo_d)[l]
                nc.sync.dma_start(
                    wt[:], src[nb].rearrange("ns p k n -> p ns k n"))
                return wt

            def load_w_chunk(wt, l, hl, nb, ns):
                src = (Whi_d if hl == 0 else Wlo_d)[l]
                nc.sync.dma_start(wt[:, ns], src[nb, ns])

            # --- startup: critical-path-ordered loads -------------------
            # First chain (panel 0, ns0, m0) consumes x-m0 + W(0,0) ns0 in
            # product order hi*hi, hi*lo, lo*hi; interleave loads in exactly
            # the consumption order so the PE starts ~3us in and never
            # starves for long. x-m1 and panel (0,1) follow.
            w00_hi = w_tile(0, 0, 0)
            w00_lo = w_tile(0, 1, 0)
            load_x_chunk(0, 0, 0, 4)
            load_w_chunk(w00_hi, 0, 0, 0, 0)
            load_x_chunk(0, 0, 4, 10)
            load_w_chunk(w00_lo, 0, 1, 0, 0)
            load_x_chunk(0, 0, 10, 16)
            load_x_chunk(1, 0, 0, 8)
            load_x_chunk(1, 0, 8, 16)
            for ns in range(1, NSB):
                load_w_chunk(w00_hi, 0, 0, 0, ns)
                load_w_chunk(w00_lo, 0, 1, 0, ns)
            for l in range(3):
                nc.sync.dma_start(
                    bias[:, l * KT:(l + 1) * KT],
                    bs[l][:].rearrange("(c p) -> p c", p=P))
            load_x(0, 1)
            load_x(1, 1)

            # --- panel schedule with one-panel weight prefetch ----------
            panels = [(l, nb) for l in range(3) for nb in range(NPB)]
            slabs = {(0, 0): (w00_hi, w00_lo)}
            for idx, (l, nb) in enumerate(panels):
                if idx + 1 < len(panels):
                    ln, nbn = panels[idx + 1]
                    slabs[(ln, nbn)] = (load_w(ln, 0, nbn),
                                        load_w(ln, 1, nbn))
                whi_t, wlo_t = slabs.pop((l, nb))
                act_in = acts[l % 2]
                act_out = acts[(l + 1) % 2] if l < 2 else None
                last_panel = idx == len(panels) - 1
                # product order matches startup DMA arrival order
                prods = [(whi_t, 0), (wlo_t, 0), (whi_t, 1)]

                z_ts, a_ts = {}, {}
                for ns in range(NSB):
                    ni = nb * NSB + ns
                    z_ts[ns] = zpool.tile([P, BL], f16,
                                          name=f"z_{l}_{ni}", tag="z16")
                    a_ts[ns] = apool.tile([P, BL], f16,
                                          name=f"a_{l}_{ni}", tag="a16")

                # first panel: m-outer so the m1 chains (whose x arrives
                # last) run after all four m0 chains; last panel: quarter
                # blocks so the tail drains in 256-col steps
                if idx == 0:
                    order = [(ns, m * MBLK, MBLK)
                             for m in range(MT) for ns in range(NSB)]
                elif last_panel:
                    order = ([(ns, m * MBLK, MBLK)
                              for ns in range(NSB - 1) for m in range(MT)]
                             + [(NSB - 1, q * (MBLK // 2), MBLK // 2)
                                for q in range(2 * MT)])
                else:
                    order = [(ns, q * (MBLK // 2), MBLK // 2)
                             for ns in range(NSB) for q in range(2 * MT)]

                for ns, ms, mw in order:
                    ni = nb * NSB + ns
                    bcol = bias[:, l * KT + ni:l * KT + ni + 1]
                    z_t, a_t = z_ts[ns], a_ts[ns]
                    psum = psp.tile([P, MBLK], f32,
                                    name=f"ps_{l}_{ni}_{ms}", tag="ps")
                    n_mm = 3 * (KT // 2)
                    i = 0
                    for wt, h in prods:
                        for jp in range(KT // 2):
                            nc.tensor.matmul(
                                psum[:, :mw],
                                wt[:, ns, 2 * jp:2 * jp + 2, :],
                                act_in[h][:, 2 * jp:2 * jp + 2,
                                          ms:ms + mw],
                                start=(i == 0),
                                stop=(i == n_mm - 1),
                                perf_mode=DR)
                            i += 1
                    nc.vector.tensor_scalar(
                        z_t[:, ms:ms + mw], psum[:, :mw],
                        1.0 / SCALE, bcol, MULT, ADD)
                    nc.scalar.activation(
                        a_t[:, ms:ms + mw], psum[:, :mw], TANH,
                        bias=bcol, scale=1.0 / SCALE)
                    if last_panel:
                        # tail: store blocks immediately. ns0-2 ride the
                        # Pool SWDGE path (no HWDGE slot), keeping HWDGE
                        # free so the final ns3 stores launch with minimal
                        # latency on the ACT/SP hardware-DGE queues.
                        if ns < NSB - 1:
                            nc.gpsimd.dma_start(
                                zouts[l][ni * P:(ni + 1) * P, ms:ms + mw],
                                z_t[:, ms:ms + mw])
                            nc.gpsimd.dma_start(
                                aouts[l][ni * P:(ni + 1) * P, ms:ms + mw],
                                a_t[:, ms:ms + mw])
                        else:
                            nc.scalar.dma_start(
                                zouts[l][ni * P:(ni + 1) * P, ms:ms + mw],
                                z_t[:, ms:ms + mw])
                            nc.sync.dma_start(
                                aouts[l][ni * P:(ni + 1) * P, ms:ms + mw],
                                a_t[:, ms:ms + mw])
                        continue
                    if ms + mw == BL:
                        # next-layer hi/lo casts first (they gate the next
                        # layer's matmuls), then stores — all on the ACT
                        # queue, ordered so every DMA wait is satisfied at
                        # decode time
                        if act_out is not None:
                            nc.scalar.activation(act_out[0][:, ni, :],
                                                 a_t[:], COPY)
                            nc.vector.tensor_tensor(act_out[1][:, ni, :],
                                                    a_t[:],
                                                    act_out[0][:, ni, :],
                                                    SUB)
                        nc.scalar.dma_start(
                            aouts[l][ni * P:(ni + 1) * P, :], a_t[:])
                        nc.scalar.dma_start(
                            zouts[l][ni * P:(ni + 1) * P, :], z_t[:])

    nc.finalize()
    return nc


_NC_CACHE = None


def _get_nc():
    global _NC_CACHE
    if _NC_CACHE is None:
        _NC_CACHE = build_nc()
    return _NC_CACHE


def _hilo(v):
    hi = v.astype(E4)
    lo = (v - hi.astype(np.float32)).astype(E4)
    return hi, lo


def _w_layout(w8):
    # [D, D] -> [nb, ns, p, k, n] with W[128k+p, 512nb+128ns+n]
    return np.ascontiguousarray(
        w8.reshape(KT, P, NPB, NSB, P).transpose(2, 3, 1, 0, 4))


def make_in_maps(x, W0, b0, W1, b1, W2, b2):
    weights = {}
    for l, W in enumerate((W0, W1, W2)):
        hi, lo = _hilo(np.asarray(W, dtype=np.float32) * SCALE)
        weights[f"Whi{l}"] = _w_layout(hi)
        weights[f"Wlo{l}"] = _w_layout(lo)
    for l, b in enumerate((b0, b1, b2)):
        weights[f"b{l}"] = np.asarray(b, dtype=np.float32)
    x = np.asarray(x, dtype=np.float32)
    in_maps = []
    for c in range(NCORES):
        xT = np.ascontiguousarray(x[c * BL:(c + 1) * BL, :].T)
        xhi, xlo = _hilo(xT)
        in_maps.append({"xhi": xhi, "xlo": xlo, **weights})
    return in_maps


def kernel(x, W0, b0, W1, b1, W2, b2):
    in_maps = make_in_maps(x, W0, b0, W1, b1, W2, b2)
    res = run_bass_kernel_spmd(_get_nc(), in_maps, core_ids=list(range(NCORES)))

    out = np.empty((6, B, D), dtype=np.float32)
    for c in range(NCORES):
        r = res.results[c]
        rows = slice(c * BL, (c + 1) * BL)
        for l in range(3):
            out[l, rows, :] = r[f"z{l}T"].astype(np.float32).T
            out[3 + l, rows, :] = r[f"a{l}T"].astype(np.float32).T
    return out


# revision 40
# speedup vs baseline: 1.0683x; 1.0060x over previous
"""DANet 3-layer MLP (B=8192, D=2048) on 8 Trainium2 NeuronCores.

Data-parallel: each core computes 1024 batch rows; weights replicated.
On-device layout is transposed (features on SBUF partitions) so every
matmul contracts over the partition dim and activations chain between
layers without transposes.

Arithmetic runs as fp8-e4m3 DoubleRow matmuls (2 packed contraction rows
per PE cell, half-rate per output column) with a hi/lo split for
accuracy: operands are decomposed v = hi + lo with hi = fp8(v) and
lo = fp8(v - hi), and each layer computes

    psum = x_hi @ W_hi + x_lo @ W_hi + x_hi @ W_lo       (W scaled by 32)

which carries ~fp16-level precision at 0.75x the PE cost of an exact
f32r chain (24 DoubleRow matmuls per 128x512 output tile vs 16 full-rate
f32r matmuls). The dropped lo*lo term is second order (~2^-13).

Weights are pre-quantized to fp8 hi/lo on the host and laid out in DRAM
as [panel][subblock][partition][k][128] so both full-panel slabs and the
startup-critical per-subblock chunks load with 2 KiB contiguous runs.
Activations are produced on-device: ACT computes a16 = tanh(psum/32 + b)
in fp16, ACT casts a_hi = fp8(a16), DVE computes a_lo = fp8(a16 - a_hi).
z/a outputs are stored as fp16 (half the store traffic) and upcast on
the host. Input x ships as fp8 hi/lo pairs (quarter traffic of f32).

Schedule: weight loads prefetch one panel ahead on the SP queue (which
never waits on compute); stores and casts ride the ACT queue ordered so
every DMA wait is already satisfied at decode time. The first panel runs
m-outer with k-chunked x loads so the PE starts ~4us into the kernel.
Steady-state chains use 256-col blocks (53ns/matmul vs 107 for 512 after
integer rounding of the per-instruction delay) across 8 PSUM banks; the
last panel's bulk stores ride the Pool SWDGE path to keep HWDGE free so
the final stores launch with minimal latency.
"""

import numpy as np
import ml_dtypes

import concourse.mybir as mybir
import concourse.tile as tile
from concourse import bacc
from concourse.bass_utils import run_bass_kernel_spmd

NCORES = 8
B = 8192
D = 2048
BL = B // NCORES          # 1024 batch rows per core
P = 128                   # partitions
KT = D // P               # 16 contraction subtiles
NPANEL = 512              # weight-panel width (output features per panel)
NPB = D // NPANEL         # 4 panels per layer
NSB = NPANEL // P         # 4 output-feature subblocks per panel
MBLK = 512                # moving-operand width (batch cols per matmul)
MT = BL // MBLK           # 2 batch blocks
SCALE = 32.0              # weight pre-scale so fp8(W*32) uses normal range

f32 = mybir.dt.float32
f16 = mybir.dt.float16
fp8 = mybir.dt.float8e4
DR = mybir.MatmulPerfMode.DoubleRow
TANH = mybir.ActivationFunctionType.Tanh
COPY = mybir.ActivationFunctionType.Copy
MULT = mybir.AluOpType.mult
ADD = mybir.AluOpType.add
SUB = mybir.AluOpType.subtract
E4 = ml_dtypes.float8_e4m3fn

W_BUFS = 6                # weight slab pool ([128,4,16,128] fp8, 8KB/partition)


def build_nc():
    nc = bacc.Bacc()

    xhi_d = nc.dram_tensor("xhi", [D, BL], fp8, kind="ExternalInput")
    xlo_d = nc.dram_tensor("xlo", [D, BL], fp8, kind="ExternalInput")
    # [nb, ns, p, k, n]: W[128k+p, 512nb+128ns+n] * 32, quantized hi/lo
    Whi_d = [nc.dram_tensor(f"Whi{l}", [NPB, NSB, P, KT, P], fp8,
                            kind="ExternalInput") for l in range(3)]
    Wlo_d = [nc.dram_tensor(f"Wlo{l}", [NPB, NSB, P, KT, P], fp8,
                            kind="ExternalInput") for l in range(3)]
    bs = [nc.dram_tensor(f"b{l}", [D], f32, kind="ExternalInput")
          for l in range(3)]
    zouts = [nc.dram_tensor(f"z{l}T", [D, BL], f16, kind="ExternalOutput")
             for l in range(3)]
    aouts = [nc.dram_tensor(f"a{l}T", [D, BL], f16, kind="ExternalOutput")
             for l in range(3)]

    with tile.TileContext(nc) as tc:
        with (
            tc.tile_pool(name="acts", bufs=1) as actp,
            tc.tile_pool(name="wpool", bufs=W_BUFS) as wpool,
            tc.tile_pool(name="zpool", bufs=4) as zpool,
            tc.tile_pool(name="apool", bufs=4) as apool,
            tc.tile_pool(name="misc", bufs=1) as misc,
            tc.tile_pool(name="psum", bufs=8, space="PSUM") as psp,
        ):
            # Persistent ping-pong activations, transposed fp8 hi/lo:
            # acts[s][h][p, k, c] = component h of a[feature 128k+p, col c].
            acts = [
                [actp.tile([P, KT, BL], fp8, name=f"act{s}_{h}", tag=f"act{s}_{h}")
                 for h in range(2)]
                for s in range(2)
            ]

            # All three biases in one [128, 48] tile; column l*16+c holds
            # b_l[128c : 128c+128].
            bias = misc.tile([P, 3 * KT], f32, name="bias", tag="bias")

            def load_x(h, m):
                ms = m * MBLK
                src = xhi_d if h == 0 else xlo_d
                nc.sync.dma_start(
                    acts[0][h][:, :, ms:ms + MBLK],
                    src[:, ms:ms + MBLK].rearrange("(k p) c -> p k c", p=P))

            def load_x_chunk(h, m, k0, k1):
                # k-subtiles [k0, k1) of batch half m — startup granularity
                ms = m * MBLK
                src = xhi_d if h == 0 else xlo_d
                nc.sync.dma_start(
                    acts[0][h][:, k0:k1, ms:ms + MBLK],
                    src[k0 * P:k1 * P, ms:ms + MBLK]
                    .rearrange("(k p) c -> p k c", p=P))

            def w_tile(l, hl, nb):
                return wpool.tile([P, NSB, KT, P], fp8,
                                  name=f"w_{l}_{hl}_{nb}", tag="wb")

            def load_w(l, hl, nb):
                wt = w_tile(l, hl, nb)
                src = (Whi_d if hl == 0 else Wlo_d)[l]
                nc.sync.dma_start(
                    wt[:], src[nb].rearrange("ns p k n -> p ns k n"))
                return wt

            def load_w_chunk(wt, l, hl, nb, ns):
                src = (Whi_d if hl == 0 else Wlo_d)[l]
                nc.sync.dma_start(wt[:, ns], src[nb, ns])

            # --- startup: critical-path-ordered loads -------------------
            # First chain (panel 0, ns0, m0) consumes x-m0 + W(0,0) ns0 in
            # product order hi*hi, hi*lo, lo*hi; interleave loads in exactly
            # the consumption order so the PE starts ~3us in and never
            # starves for long. x-m1 and panel (0,1) follow.
            w00_hi = w_tile(0, 0, 0)
            w00_lo = w_tile(0, 1, 0)
            load_x_chunk(0, 0, 0, 4)
            load_w_chunk(w00_hi, 0, 0, 0, 0)
            load_x_chunk(0, 0, 4, 10)
            load_w_chunk(w00_lo, 0, 1, 0, 0)
            load_x_chunk(0, 0, 10, 16)
            load_x_chunk(1, 0, 0, 8)
            load_x_chunk(1, 0, 8, 16)
            for ns in range(1, NSB):
                load_w_chunk(w00_hi, 0, 0, 0, ns)
                load_w_chunk(w00_lo, 0, 1, 0, ns)
            load_x(0, 1)
            load_x(1, 1)
            # bias loads ride at the tail of startup: their 3 HWDGE slots
            # would otherwise delay the x-m1 transfers that gate the first
            # panel's m1 chains; the DVE-z consumers that need the bias are
            # absorbed by the 8 in-flight PSUM banks until it lands
            for l in range(3):
                nc.sync.dma_start(
                    bias[:, l * KT:(l + 1) * KT],
                    bs[l][:].rearrange("(c p) -> p c", p=P))

            # --- panel schedule with one-panel weight prefetch ----------
            panels = [(l, nb) for l in range(3) for nb in range(NPB)]
            slabs = {(0, 0): (w00_hi, w00_lo)}
            for idx, (l, nb) in enumerate(panels):
                if idx + 1 < len(panels):
                    ln, nbn = panels[idx + 1]
                    slabs[(ln, nbn)] = (load_w(ln, 0, nbn),
                                        load_w(ln, 1, nbn))
                whi_t, wlo_t = slabs.pop((l, nb))
                act_in = acts[l % 2]
                act_out = acts[(l + 1) % 2] if l < 2 else None
                last_panel = idx == len(panels) - 1
                # product order matches startup DMA arrival order
                prods = [(whi_t, 0), (wlo_t, 0), (whi_t, 1)]

                z_ts, a_ts = {}, {}
                for ns in range(NSB):
                    ni = nb * NSB + ns
                    z_ts[ns] = zpool.tile([P, BL], f16,
                                          name=f"z_{l}_{ni}", tag="z16")
                    a_ts[ns] = apool.tile([P, BL], f16,
                                          name=f"a_{l}_{ni}", tag="a16")

                # first panel: m-outer so the m1 chains (whose x arrives
                # last) run after all four m0 chains; last panel: quarter
                # blocks so the tail drains in 256-col steps
                if idx == 0:
                    order = [(ns, m * MBLK, MBLK)
                             for m in range(MT) for ns in range(NSB)]
                elif last_panel:
                    order = ([(ns, m * MBLK, MBLK)
                              for ns in range(NSB - 1) for m in range(MT)]
                             + [(NSB - 1, q * (MBLK // 2), MBLK // 2)
                                for q in range(2 * MT)])
                else:
                    order = [(ns, q * (MBLK // 2), MBLK // 2)
                             for ns in range(NSB) for q in range(2 * MT)]

                for ns, ms, mw in order:
                    ni = nb * NSB + ns
                    bcol = bias[:, l * KT + ni:l * KT + ni + 1]
                    z_t, a_t = z_ts[ns], a_ts[ns]
                    psum = psp.tile([P, MBLK], f32,
                                    name=f"ps_{l}_{ni}_{ms}", tag="ps")
                    n_mm = 3 * (KT // 2)
                    i = 0
                    for wt, h in prods:
                        for jp in range(KT // 2):
                            nc.tensor.matmul(
                                psum[:, :mw],
                                wt[:, ns, 2 * jp:2 * jp + 2, :],
                                act_in[h][:, 2 * jp:2 * jp + 2,
                                          ms:ms + mw],
                                start=(i == 0),
                                stop=(i == n_mm - 1),
                                perf_mode=DR)
                            i += 1
                    nc.vector.tensor_scalar(
                        z_t[:, ms:ms + mw], psum[:, :mw],
                        1.0 / SCALE, bcol, MULT, ADD)
                    nc.scalar.activation(
                        a_t[:, ms:ms + mw], psum[:, :mw], TANH,
                        bias=bcol, scale=1.0 / SCALE)
                    if last_panel:
                        # tail: store blocks immediately. ns0-2 ride the
                        # Pool SWDGE path (no HWDGE slot), keeping HWDGE
                        # free so the final ns3 stores launch with minimal
                        # latency on the ACT/SP hardware-DGE queues.
                        if ns < NSB - 1:
                            nc.gpsimd.dma_start(
                                zouts[l][ni * P:(ni + 1) * P, ms:ms + mw],
                                z_t[:, ms:ms + mw])
                            nc.gpsimd.dma_start(
                                aouts[l][ni * P:(ni + 1) * P, ms:ms + mw],
                                a_t[:, ms:ms + mw])
                        else:
                            nc.scalar.dma_start(
                                zouts[l][ni * P:(ni + 1) * P, ms:ms + mw],
                                z_t[:, ms:ms + mw])
                            nc.sync.dma_start(
                                aouts[l][ni * P:(ni + 1) * P, ms:ms + mw],
                                a_t[:, ms:ms + mw])
                        continue
                    if ms + mw == BL:
                        # next-layer hi/lo casts first (they gate the next
                        # layer's matmuls), then stores — all on the ACT
                        # queue, ordered so every DMA wait is satisfied at
                        # decode time
                        if act_out is not None:
                            nc.scalar.activation(act_out[0][:, ni, :],
                                                 a_t[:], COPY)
                            nc.vector.tensor_tensor(act_out[1][:, ni, :],
                                                    a_t[:],
                                                    act_out[0][:, ni, :],
                                                    SUB)
                        nc.scalar.dma_start(
                            aouts[l][ni * P:(ni + 1) * P, :], a_t[:])
                        nc.scalar.dma_start(
                            zouts[l][ni * P:(ni + 1) * P, :], z_t[:])

    nc.finalize()
    return nc


_NC_CACHE = None


def _get_nc():
    global _NC_CACHE
    if _NC_CACHE is None:
        _NC_CACHE = build_nc()
    return _NC_CACHE


def _hilo(v):
    hi = v.astype(E4)
    lo = (v - hi.astype(np.float32)).astype(E4)
    return hi, lo


def _w_layout(w8):
    # [D, D] -> [nb, ns, p, k, n] with W[128k+p, 512nb+128ns+n]
    return np.ascontiguousarray(
        w8.reshape(KT, P, NPB, NSB, P).transpose(2, 3, 1, 0, 4))


def make_in_maps(x, W0, b0, W1, b1, W2, b2):
    weights = {}
    for l, W in enumerate((W0, W1, W2)):
        hi, lo = _hilo(np.asarray(W, dtype=np.float32) * SCALE)
        weights[f"Whi{l}"] = _w_layout(hi)
        weights[f"Wlo{l}"] = _w_layout(lo)
    for l, b in enumerate((b0, b1, b2)):
        weights[f"b{l}"] = np.asarray(b, dtype=np.float32)
    x = np.asarray(x, dtype=np.float32)
    in_maps = []
    for c in range(NCORES):
        xT = np.ascontiguousarray(x[c * BL:(c + 1) * BL, :].T)
        xhi, xlo = _hilo(xT)
        in_maps.append({"xhi": xhi, "xlo": xlo, **weights})
    return in_maps


def kernel(x, W0, b0, W1, b1, W2, b2):
    in_maps = make_in_maps(x, W0, b0, W1, b1, W2, b2)
    res = run_bass_kernel_spmd(_get_nc(), in_maps, core_ids=list(range(NCORES)))

    out = np.empty((6, B, D), dtype=np.float32)
    for c in range(NCORES):
        r = res.results[c]
        rows = slice(c * BL, (c + 1) * BL)
        for l in range(3):
            out[l, rows, :] = r[f"z{l}T"].astype(np.float32).T
            out[3 + l, rows, :] = r[f"a{l}T"].astype(np.float32).T
    return out


# revision 41
# speedup vs baseline: 1.0689x; 1.0006x over previous
"""DANet 3-layer MLP (B=8192, D=2048) on 8 Trainium2 NeuronCores.

Data-parallel: each core computes 1024 batch rows; weights replicated.
On-device layout is transposed (features on SBUF partitions) so every
matmul contracts over the partition dim and activations chain between
layers without transposes.

Arithmetic runs as fp8-e4m3 DoubleRow matmuls (2 packed contraction rows
per PE cell, half-rate per output column) with a hi/lo split for
accuracy: operands are decomposed v = hi + lo with hi = fp8(v) and
lo = fp8(v - hi), and each layer computes

    psum = x_hi @ W_hi + x_lo @ W_hi + x_hi @ W_lo       (W scaled by 32)

which carries ~fp16-level precision at 0.75x the PE cost of an exact
f32r chain (24 DoubleRow matmuls per 128x512 output tile vs 16 full-rate
f32r matmuls). The dropped lo*lo term is second order (~2^-13).

Weights are pre-quantized to fp8 hi/lo on the host and laid out in DRAM
as [panel][subblock][partition][k][128] so both full-panel slabs and the
startup-critical per-subblock chunks load with 2 KiB contiguous runs.
Activations are produced on-device: ACT computes a16 = tanh(psum/32 + b)
in fp16, ACT casts a_hi = fp8(a16), DVE computes a_lo = fp8(a16 - a_hi).
z/a outputs are stored as fp16 (half the store traffic) and upcast on
the host. Input x ships as fp8 hi/lo pairs (quarter traffic of f32).

Schedule: weight loads prefetch one panel ahead on the SP queue (which
never waits on compute); stores and casts ride the ACT queue ordered so
every DMA wait is already satisfied at decode time. The first panel runs
m-outer with k-chunked x loads so the PE starts ~4us into the kernel.
Steady-state chains use 256-col blocks (53ns/matmul vs 107 for 512 after
integer rounding of the per-instruction delay) across 8 PSUM banks; the
last panel's bulk stores ride the Pool SWDGE path to keep HWDGE free so
the final stores launch with minimal latency.
"""

import numpy as np
import ml_dtypes

import concourse.mybir as mybir
import concourse.tile as tile
from concourse import bacc
from concourse.bass_utils import run_bass_kernel_spmd

NCORES = 8
B = 8192
D = 2048
BL = B // NCORES          # 1024 batch rows per core
P = 128                   # partitions
KT = D // P               # 16 contraction subtiles
NPANEL = 512              # weight-panel width (output features per panel)
NPB = D // NPANEL         # 4 panels per layer
NSB = NPANEL // P         # 4 output-feature subblocks per panel
MBLK = 512                # moving-operand width (batch cols per matmul)
MT = BL // MBLK           # 2 batch blocks
SCALE = 32.0              # weight pre-scale so fp8(W*32) uses normal range

f32 = mybir.dt.float32
f16 = mybir.dt.float16
fp8 = mybir.dt.float8e4
DR = mybir.MatmulPerfMode.DoubleRow
TANH = mybir.ActivationFunctionType.Tanh
COPY = mybir.ActivationFunctionType.Copy
MULT = mybir.AluOpType.mult
ADD = mybir.AluOpType.add
SUB = mybir.AluOpType.subtract
E4 = ml_dtypes.float8_e4m3fn

W_BUFS = 6                # weight slab pool ([128,4,16,128] fp8, 8KB/partition)


def build_nc():
    nc = bacc.Bacc()

    xhi_d = nc.dram_tensor("xhi", [D, BL], fp8, kind="ExternalInput")
    xlo_d = nc.dram_tensor("xlo", [D, BL], fp8, kind="ExternalInput")
    # [nb, ns, p, k, n]: W[128k+p, 512nb+128ns+n] * 32, quantized hi/lo
    Whi_d = [nc.dram_tensor(f"Whi{l}", [NPB, NSB, P, KT, P], fp8,
                            kind="ExternalInput") for l in range(3)]
    Wlo_d = [nc.dram_tensor(f"Wlo{l}", [NPB, NSB, P, KT, P], fp8,
                            kind="ExternalInput") for l in range(3)]
    bs = [nc.dram_tensor(f"b{l}", [D], f32, kind="ExternalInput")
          for l in range(3)]
    zouts = [nc.dram_tensor(f"z{l}T", [D, BL], f16, kind="ExternalOutput")
             for l in range(3)]
    aouts = [nc.dram_tensor(f"a{l}T", [D, BL], f16, kind="ExternalOutput")
             for l in range(3)]

    with tile.TileContext(nc) as tc:
        with (
            tc.tile_pool(name="acts", bufs=1) as actp,
            tc.tile_pool(name="wpool", bufs=W_BUFS) as wpool,
            tc.tile_pool(name="zpool", bufs=4) as zpool,
            tc.tile_pool(name="apool", bufs=4) as apool,
            tc.tile_pool(name="misc", bufs=1) as misc,
            tc.tile_pool(name="psum", bufs=8, space="PSUM") as psp,
        ):
            # Persistent ping-pong activations, transposed fp8 hi/lo:
            # acts[s][h][p, k, c] = component h of a[feature 128k+p, col c].
            acts = [
                [actp.tile([P, KT, BL], fp8, name=f"act{s}_{h}", tag=f"act{s}_{h}")
                 for h in range(2)]
                for s in range(2)
            ]

            # All three biases in one [128, 48] tile; column l*16+c holds
            # b_l[128c : 128c+128].
            bias = misc.tile([P, 3 * KT], f32, name="bias", tag="bias")

            def load_x(h, m):
                ms = m * MBLK
                src = xhi_d if h == 0 else xlo_d
                nc.sync.dma_start(
                    acts[0][h][:, :, ms:ms + MBLK],
                    src[:, ms:ms + MBLK].rearrange("(k p) c -> p k c", p=P))

            def load_x_chunk(h, m, k0, k1):
                # k-subtiles [k0, k1) of batch half m — startup granularity
                ms = m * MBLK
                src = xhi_d if h == 0 else xlo_d
                nc.sync.dma_start(
                    acts[0][h][:, k0:k1, ms:ms + MBLK],
                    src[k0 * P:k1 * P, ms:ms + MBLK]
                    .rearrange("(k p) c -> p k c", p=P))

            def w_tile(l, hl, nb):
                return wpool.tile([P, NSB, KT, P], fp8,
                                  name=f"w_{l}_{hl}_{nb}", tag="wb")

            def load_w(l, hl, nb):
                wt = w_tile(l, hl, nb)
                src = (Whi_d if hl == 0 else Wlo_d)[l]
                nc.sync.dma_start(
                    wt[:], src[nb].rearrange("ns p k n -> p ns k n"))
                return wt

            def load_w_chunk(wt, l, hl, nb, ns):
                src = (Whi_d if hl == 0 else Wlo_d)[l]
                nc.sync.dma_start(wt[:, ns], src[nb, ns])

            # --- startup: critical-path-ordered loads -------------------
            # First chain (panel 0, ns0, m0) consumes x-m0 + W(0,0) ns0 in
            # product order hi*hi, hi*lo, lo*hi; interleave loads in exactly
            # the consumption order so the PE starts ~3us in and never
            # starves for long. x-m1 and panel (0,1) follow.
            w00_hi = w_tile(0, 0, 0)
            w00_lo = w_tile(0, 1, 0)
            load_x_chunk(0, 0, 0, 4)
            load_w_chunk(w00_hi, 0, 0, 0, 0)
            load_x_chunk(0, 0, 4, 10)
            load_w_chunk(w00_lo, 0, 1, 0, 0)
            load_x_chunk(0, 0, 10, 16)
            load_x_chunk(1, 0, 0, 8)
            load_x_chunk(1, 0, 8, 16)
            for ns in range(1, NSB):
                load_w_chunk(w00_hi, 0, 0, 0, ns)
                load_w_chunk(w00_lo, 0, 1, 0, ns)
            load_x(0, 1)
            load_x(1, 1)
            # bias loads ride at the tail of startup: their 3 HWDGE slots
            # would otherwise delay the x-m1 transfers that gate the first
            # panel's m1 chains; the DVE-z consumers that need the bias are
            # absorbed by the 8 in-flight PSUM banks until it lands
            for l in range(3):
                nc.sync.dma_start(
                    bias[:, l * KT:(l + 1) * KT],
                    bs[l][:].rearrange("(c p) -> p c", p=P))

            # --- panel schedule with one-panel weight prefetch ----------
            panels = [(l, nb) for l in range(3) for nb in range(NPB)]
            slabs = {(0, 0): (w00_hi, w00_lo)}
            for idx, (l, nb) in enumerate(panels):
                if idx + 1 < len(panels):
                    ln, nbn = panels[idx + 1]
                    slabs[(ln, nbn)] = (load_w(ln, 0, nbn),
                                        load_w(ln, 1, nbn))
                whi_t, wlo_t = slabs.pop((l, nb))
                act_in = acts[l % 2]
                act_out = acts[(l + 1) % 2] if l < 2 else None
                last_panel = idx == len(panels) - 1
                # product order matches startup DMA arrival order
                prods = [(whi_t, 0), (wlo_t, 0), (whi_t, 1)]

                z_ts, a_ts = {}, {}
                for ns in range(NSB):
                    ni = nb * NSB + ns
                    z_ts[ns] = zpool.tile([P, BL], f16,
                                          name=f"z_{l}_{ni}", tag="z16")
                    a_ts[ns] = apool.tile([P, BL], f16,
                                          name=f"a_{l}_{ni}", tag="a16")

                # first panel: m-outer so the m1 chains (whose x arrives
                # last) run after all four m0 chains; last panel: quarter
                # blocks so the tail drains in 256-col steps
                if idx == 0:
                    order = [(ns, m * MBLK, MBLK)
                             for m in range(MT) for ns in range(NSB)]
                elif last_panel:
                    order = [(ns, q * (MBLK // 2), MBLK // 2)
                             for ns in range(NSB) for q in range(2 * MT)]
                else:
                    order = [(ns, q * (MBLK // 2), MBLK // 2)
                             for ns in range(NSB) for q in range(2 * MT)]

                for ns, ms, mw in order:
                    ni = nb * NSB + ns
                    bcol = bias[:, l * KT + ni:l * KT + ni + 1]
                    z_t, a_t = z_ts[ns], a_ts[ns]
                    psum = psp.tile([P, MBLK], f32,
                                    name=f"ps_{l}_{ni}_{ms}", tag="ps")
                    n_mm = 3 * (KT // 2)
                    i = 0
                    for wt, h in prods:
                        for jp in range(KT // 2):
                            nc.tensor.matmul(
                                psum[:, :mw],
                                wt[:, ns, 2 * jp:2 * jp + 2, :],
                                act_in[h][:, 2 * jp:2 * jp + 2,
                                          ms:ms + mw],
                                start=(i == 0),
                                stop=(i == n_mm - 1),
                                perf_mode=DR)
                            i += 1
                    nc.vector.tensor_scalar(
                        z_t[:, ms:ms + mw], psum[:, :mw],
                        1.0 / SCALE, bcol, MULT, ADD)
                    nc.scalar.activation(
                        a_t[:, ms:ms + mw], psum[:, :mw], TANH,
                        bias=bcol, scale=1.0 / SCALE)
                    if last_panel:
                        # tail: store blocks immediately. ns0-2 ride the
                        # Pool SWDGE path (no HWDGE slot), keeping HWDGE
                        # free so the final ns3 stores launch with minimal
                        # latency on the ACT/SP hardware-DGE queues.
                        if ns < NSB - 1:
                            if (ms + mw) % MBLK == 0:
                                hs = ms + mw - MBLK
                                nc.gpsimd.dma_start(
                                    zouts[l][ni * P:(ni + 1) * P,
                                             hs:hs + MBLK],
                                    z_t[:, hs:hs + MBLK])
                                nc.gpsimd.dma_start(
                                    aouts[l][ni * P:(ni + 1) * P,
                                             hs:hs + MBLK],
                                    a_t[:, hs:hs + MBLK])
                        else:
                            nc.scalar.dma_start(
                                zouts[l][ni * P:(ni + 1) * P, ms:ms + mw],
                                z_t[:, ms:ms + mw])
                            nc.sync.dma_start(
                                aouts[l][ni * P:(ni + 1) * P, ms:ms + mw],
                                a_t[:, ms:ms + mw])
                        continue
                    if ms + mw == BL:
                        # next-layer hi/lo casts first (they gate the next
                        # layer's matmuls), then stores — all on the ACT
                        # queue, ordered so every DMA wait is satisfied at
                        # decode time
                        if act_out is not None:
                            nc.scalar.activation(act_out[0][:, ni, :],
                                                 a_t[:], COPY)
                            nc.vector.tensor_tensor(act_out[1][:, ni, :],
                                                    a_t[:],
                                                    act_out[0][:, ni, :],
                                                    SUB)
                        nc.scalar.dma_start(
                            aouts[l][ni * P:(ni + 1) * P, :], a_t[:])
                        nc.scalar.dma_start(
                            zouts[l][ni * P:(ni + 1) * P, :], z_t[:])

    nc.finalize()
    return nc


_NC_CACHE = None


def _get_nc():
    global _NC_CACHE
    if _NC_CACHE is None:
        _NC_CACHE = build_nc()
    return _NC_CACHE


def _hilo(v):
    hi = v.astype(E4)
    lo = (v - hi.astype(np.float32)).astype(E4)
    return hi, lo


def _w_layout(w8):
    # [D, D] -> [nb, ns, p, k, n] with W[128k+p, 512nb+128ns+n]
    return np.ascontiguousarray(
        w8.reshape(KT, P, NPB, NSB, P).transpose(2, 3, 1, 0, 4))


def make_in_maps(x, W0, b0, W1, b1, W2, b2):
    weights = {}
    for l, W in enumerate((W0, W1, W2)):
        hi, lo = _hilo(np.asarray(W, dtype=np.float32) * SCALE)
        weights[f"Whi{l}"] = _w_layout(hi)
        weights[f"Wlo{l}"] = _w_layout(lo)
    for l, b in enumerate((b0, b1, b2)):
        weights[f"b{l}"] = np.asarray(b, dtype=np.float32)
    x = np.asarray(x, dtype=np.float32)
    in_maps = []
    for c in range(NCORES):
        xT = np.ascontiguousarray(x[c * BL:(c + 1) * BL, :].T)
        xhi, xlo = _hilo(xT)
        in_maps.append({"xhi": xhi, "xlo": xlo, **weights})
    return in_maps


def kernel(x, W0, b0, W1, b1, W2, b2):
    in_maps = make_in_maps(x, W0, b0, W1, b1, W2, b2)
    res = run_bass_kernel_spmd(_get_nc(), in_maps, core_ids=list(range(NCORES)))

    out = np.empty((6, B, D), dtype=np.float32)
    for c in range(NCORES):
        r = res.results[c]
        rows = slice(c * BL, (c + 1) * BL)
        for l in range(3):
            out[l, rows, :] = r[f"z{l}T"].astype(np.float32).T
            out[3 + l, rows, :] = r[f"a{l}T"].astype(np.float32).T
    return out
